# revision 6
# baseline (speedup 1.0000x reference)
"""Capsule-FC dynamic-routing kernel for 8 Trainium2 NeuronCores.

Math (reference):
    u[b,i,j,o] = sum_d W[i,j,o,d] * x[b,i,d]          (never materialized here)
    b=0; 3x: c = softmax(b, j); s = squash(sum_i c*u); b += sum_b <u, s>

Distribution: data-parallel over batch B=256 -> 32 per core; W replicated.
The [I,J] agreement is AllReduce-summed across cores each routing iter
(the last iteration needs no b update, so only 2 AllReduces).

Per-core algorithm (u-free formulation):
    s[b,(j,o)]   = sum_{(i,d)} (c[i,j]*W[i,(j,o),d]) * x[b,(i,d)]     (PE, K=(i,d))
    T[(i,d),(j,o)] = sum_b x[b,(i,d)] * s[b,(j,o)]                    (PE, K=b, row-tiled)
    A[i,j]       = sum_{d,o} W'[(i,d),(j,o)] * T[(i,d),(j,o)]         (DVE mult + o-reduce, PE d-reduce)

Precision: x and cW are used as hi/lo bf16 pairs with three bf16 matmul
terms (hh + hl + lh), f32 PSUM accumulation; V/A path in f32. Validated
host-side at 3.9e-3 absmax-rel vs the f32 reference (gate 2e-2).
"""

import os
import sys

import numpy as np
import ml_dtypes

for _p in ("/opt/trn_rl_repo", "/opt/pypackages"):
    if _p not in sys.path:
        sys.path.insert(0, _p)

import concourse.bass as bass
import concourse.bacc as bacc
import concourse.tile as tile
import concourse.mybir as mybir

B, I, J, DIN, DOUT = 256, 1152, 10, 8, 16
NCORES = 8
BL = B // NCORES          # 32 local batch
ID = I * DIN              # 9216 = (i,d)
JO = J * DOUT             # 160 = (j,o)
NCHUNK = ID // 128        # 72 chunks of 128 (i,d) rows; chunk cc holds i in [16cc,16cc+16)
NCB = I // 128            # 9  i-blocks of 128 for b/c logits layout
GRP = 3                   # T/V chunks per PSUM bank group
NGRP = NCHUNK // GRP      # 24
ITERS = 3

BF = mybir.dt.bfloat16
F32 = mybir.dt.float32
AX = mybir.AxisListType
AF = mybir.ActivationFunctionType

LAST_EXEC_NS = None

# Row-tiled T-matmuls (tile_position): 0 = off, N = rotate over N row
# groups (positions 0/32/64/96). (96,0) faulted on HW; 3 keeps 0/32/64.
ROW_TILE = int(os.environ.get("CAPS_ROW_TILE", "0"))

_CACHE = {}


def _bf16(a):
    return a.astype(ml_dtypes.bfloat16)


def build_program():
    nc = bacc.Bacc("TRN2", target_bir_lowering=False, debug=False,
                   num_devices=NCORES)

    # ---- DRAM I/O (per-core shards; names are the in_maps keys) ----
    xT_h = nc.dram_tensor("xT_h", [ID, BL], BF, kind="ExternalInput")
    xT_l = nc.dram_tensor("xT_l", [ID, BL], BF, kind="ExternalInput")
    xF_h = nc.dram_tensor("xF_h", [128, ID], BF, kind="ExternalInput")
    xF_l = nc.dram_tensor("xF_l", [128, ID], BF, kind="ExternalInput")
    Wp32 = nc.dram_tensor("Wp32", [ID, JO], F32, kind="ExternalInput")
    Wp_h = nc.dram_tensor("Wp_h", [ID, JO], BF, kind="ExternalInput")
    Wp_l = nc.dram_tensor("Wp_l", [ID, JO], BF, kind="ExternalInput")
    sel = nc.dram_tensor("sel", [8, 128, 128], BF, kind="ExternalInput")
    selR = nc.dram_tensor("selR", [128, 16], F32, kind="ExternalInput")
    rep4 = nc.dram_tensor("rep4", [BL, 128], BF, kind="ExternalInput")
    out_s = nc.dram_tensor("out_s", [BL, JO], F32, kind="ExternalOutput")

    with tile.TileContext(nc) as tc:
        with (
            tc.tile_pool(name="wide", bufs=1) as wide,
            tc.tile_pool(name="small", bufs=2) as small,
            tc.tile_pool(name="vpool", bufs=3) as vpool,
            tc.tile_pool(name="ps_s", bufs=1, space="PSUM") as ps_s,
            tc.tile_pool(name="ps_T", bufs=4, space="PSUM") as ps_T,
            tc.tile_pool(name="ps_x", bufs=1, space="PSUM") as ps_x,
            tc.tile_pool(name="ps_a", bufs=1, space="PSUM") as ps_a,
            tc.tile_pool(name="dram", bufs=1, space="DRAM") as dram,
        ):
            # ---- persistent SBUF residents ----
            xTh_sb = wide.tile([128, NCHUNK * BL], BF, tag="xTh")
            xTl_sb = wide.tile([128, NCHUNK * BL], BF, tag="xTl")
            xFh_sb = wide.tile([128, ID], BF, tag="xFh")
            xFl_sb = wide.tile([128, ID], BF, tag="xFl")
            W32_sb = wide.tile([128, NCHUNK * JO], F32, tag="W32")
            Wh_sb = wide.tile([128, NCHUNK * JO], BF, tag="Wh")
            Wl_sb = wide.tile([128, NCHUNK * JO], BF, tag="Wl")
            cWh_sb = wide.tile([128, NCHUNK * JO], BF, tag="cWh")
            cWl_sb = wide.tile([128, NCHUNK * JO], BF, tag="cWl")
            sel_sb = wide.tile([128, 8 * 128], BF, tag="sel")
            selR_sb = wide.tile([128, 16], F32, tag="selR")
            rep4_sb = wide.tile([BL, 128], BF, tag="rep4")
            b_sb = wide.tile([128, NCB * J], F32, tag="b")
            cexp_sb = wide.tile([128, 8 * NCB * J], BF, tag="cexp")
            A_sb = wide.tile([16, NCHUNK * J], F32, tag="A")
            A_back = wide.tile([128, NCB * J], F32, tag="Aback")

            # DRAM bounce buffers for the collective
            A_dram = dram.tile([I, J], F32)
            A_red = dram.tile([I, J], F32)

            # ---- load everything (Tile overlaps DMAs with compute) ----
            nc.sync.dma_start(xTh_sb[:].rearrange("p (c b) -> p c b", c=NCHUNK),
                              xT_h.ap().rearrange("(c p) b -> p c b", p=128))
            nc.sync.dma_start(xTl_sb[:].rearrange("p (c b) -> p c b", c=NCHUNK),
                              xT_l.ap().rearrange("(c p) b -> p c b", p=128))
            nc.sync.dma_start(Wh_sb[:].rearrange("p (c f) -> p c f", c=NCHUNK),
                              Wp_h.ap().rearrange("(c p) f -> p c f", p=128))
            nc.sync.dma_start(Wl_sb[:].rearrange("p (c f) -> p c f", c=NCHUNK),
                              Wp_l.ap().rearrange("(c p) f -> p c f", p=128))
            nc.sync.dma_start(W32_sb[:].rearrange("p (c f) -> p c f", c=NCHUNK),
                              Wp32.ap().rearrange("(c p) f -> p c f", p=128))
            nc.sync.dma_start(xFh_sb[:], xF_h.ap())
            nc.sync.dma_start(xFl_sb[:], xF_l.ap())
            nc.sync.dma_start(sel_sb[:].rearrange("p (g m) -> p g m", g=8),
                              sel.ap().rearrange("g p m -> p g m"))
            nc.sync.dma_start(selR_sb[:], selR.ap())
            nc.sync.dma_start(rep4_sb[:], rep4.ap())

            nc.vector.memset(b_sb[:], 0.0)

            for t in range(ITERS):
                first_iter = t == 0
                last_iter = t == ITERS - 1

                # ============ phase A: softmax + c_exp spread + cW ============
                if not first_iter:
                    bv = b_sb[:].rearrange("p (c j) -> p c j", c=NCB)
                    mx = small.tile([128, NCB], F32, tag="mx")
                    nc.vector.reduce_max(out=mx[:], in_=bv, axis=AX.X)
                    ex = small.tile([128, NCB * J], F32, tag="ex")
                    exv = ex[:].rearrange("p (c j) -> p c j", c=NCB)
                    mxb = mx[:].rearrange("p (c o) -> p c o", o=1).broadcast_to(
                        (128, NCB, J))
                    nc.vector.tensor_sub(exv, bv, mxb)
                    nc.scalar.activation(ex[:], ex[:], AF.Exp)
                    zs = small.tile([128, NCB], F32, tag="zs")
                    nc.vector.reduce_sum(out=zs[:], in_=exv, axis=AX.X)
                    rz = small.tile([128, NCB], F32, tag="rz")
                    nc.vector.reciprocal(rz[:], zs[:])
                    c_sb = small.tile([128, NCB * J], BF, tag="c")
                    rzb = rz[:].rearrange("p (c o) -> p c o", o=1).broadcast_to(
                        (128, NCB, J))
                    nc.vector.tensor_mul(
                        c_sb[:].rearrange("p (c j) -> p c j", c=NCB), exv, rzb)

                    # spread c[i,j] -> c_exp[(il,d), (cb,j)] per g (i = 128cb+16g+il)
                    for g in range(8):
                        cexp_ps = ps_x.tile([128, NCB * J], F32, tag="cexp_ps")
                        nc.tensor.matmul(cexp_ps[:],
                                         sel_sb[:, g * 128:(g + 1) * 128],
                                         c_sb[:], start=True, stop=True)
                        nc.vector.tensor_copy(
                            cexp_sb[:, g * (NCB * J):(g + 1) * (NCB * J)],
                            cexp_ps[:])

                    # cW pair: chunk cc = 8*cb + g lives at free offset cc*JO
                    for g in range(8):
                        cx = cexp_sb[:, g * (NCB * J):(g + 1) * (NCB * J)]
                        cxb = cx.rearrange("p (c j o) -> p c j o", c=NCB,
                                           o=1).broadcast_to((128, NCB, J, DOUT))
                        for src, dst in ((Wh_sb, cWh_sb), (Wl_sb, cWl_sb)):
                            sv = src[:].rearrange("p (c g j o) -> p g c j o",
                                                  c=NCB, g=8, j=J)[:, g]
                            dv = dst[:].rearrange("p (c g j o) -> p g c j o",
                                                  c=NCB, g=8, j=J)[:, g]
                            nc.vector.tensor_mul(dv, sv, cxb)

                # ============ phase B: s = sum_(i,d) cW * x  (3-term hi/lo) ====
                rh_src = Wh_sb if first_iter else cWh_sb
                rl_src = Wl_sb if first_iter else cWl_sb
                s_ps = ps_s.tile([BL, JO], F32, tag="s_ps")
                for cc in range(NCHUNK):
                    lh = xTh_sb[:, cc * BL:(cc + 1) * BL]
                    ll = xTl_sb[:, cc * BL:(cc + 1) * BL]
                    rh = rh_src[:, cc * JO:(cc + 1) * JO]
                    rl = rl_src[:, cc * JO:(cc + 1) * JO]
                    nc.tensor.matmul(s_ps[:], lh, rh, start=(cc == 0), stop=False)
                    nc.tensor.matmul(s_ps[:], lh, rl, start=False, stop=False)
                    nc.tensor.matmul(s_ps[:], ll, rh, start=False,
                                     stop=(cc == NCHUNK - 1))

                # ============ squash ============
                s32 = small.tile([BL, JO], F32, tag="s32")
                nc.vector.tensor_copy(s32[:], s_ps[:])
                sq = small.tile([BL, JO], F32, tag="sq")
                nc.vector.tensor_mul(sq[:], s32[:], s32[:])
                n2 = small.tile([BL, J], F32, tag="n2")
                nc.vector.reduce_sum(out=n2[:],
                                     in_=sq[:].rearrange("p (j o) -> p j o", j=J),
                                     axis=AX.X)
                if first_iter:
                    # c was uniform 1/J=0.1 (folded out of phase B): s*=0.1 -> n2*=0.01
                    nc.vector.tensor_scalar_mul(n2[:], n2[:], 0.01)
                l2t = small.tile([BL, J], F32, tag="l2t")
                nc.scalar.activation(l2t[:], n2[:], AF.Sqrt)
                den = small.tile([BL, J], F32, tag="den")
                nc.vector.tensor_scalar_add(den[:], n2[:], 1.0)
                rden = small.tile([BL, J], F32, tag="rden")
                nc.vector.reciprocal(rden[:], den[:])
                fac = small.tile([BL, J], F32, tag="fac")
                nc.vector.tensor_mul(fac[:], l2t[:], rden[:])
                if first_iter:
                    nc.vector.tensor_scalar_mul(fac[:], fac[:], 0.1)
                s_sq = small.tile([BL, JO], F32, tag="s_sq")
                facb = fac[:].rearrange("p (j o) -> p j o", o=1).broadcast_to(
                    (BL, J, DOUT))
                nc.vector.tensor_mul(s_sq[:].rearrange("p (j o) -> p j o", j=J),
                                     s32[:].rearrange("p (j o) -> p j o", j=J),
                                     facb)

                if last_iter:
                    nc.sync.dma_start(out_s.ap(), s_sq[:])
                    continue

                # ============ phase C: T, V, A ============
                sh = small.tile([BL, JO], BF, tag="sh")
                nc.vector.tensor_copy(sh[:], s_sq[:])
                sl = small.tile([BL, JO], BF, tag="sl")
                nc.vector.tensor_sub(sl[:], s_sq[:], sh[:])
                # replicate s pair to all 4 partition groups (for row tiling)
                shr = small.tile([128, JO], BF, tag="shr")
                slr = small.tile([128, JO], BF, tag="slr")
                for src, dst in ((sh, shr), (sl, slr)):
                    rp = ps_x.tile([128, JO], F32, tag="rep_ps")
                    nc.tensor.matmul(rp[:], rep4_sb[:], src[:], start=True,
                                     stop=True)
                    nc.vector.tensor_copy(dst[:], rp[:])

                for grp in range(NGRP):
                    T_ps = ps_T.tile([128, GRP * JO], F32, tag="T_ps")
                    for k in range(GRP):
                        cc = grp * GRP + k
                        r = (cc % ROW_TILE) if ROW_TILE else 0
                        rows = slice(32 * r, 32 * (r + 1))
                        cols = slice(cc * 128, (cc + 1) * 128)
                        o = T_ps[:, k * JO:(k + 1) * JO]
                        tp = (32 * r, 0) if ROW_TILE else None
                        nc.tensor.matmul(o, xFh_sb[rows, cols], shr[rows, :],
                                         start=True, stop=False, tile_position=tp)
                        nc.tensor.matmul(o, xFh_sb[rows, cols], slr[rows, :],
                                         start=False, stop=False, tile_position=tp)
                        nc.tensor.matmul(o, xFl_sb[rows, cols], shr[rows, :],
                                         start=False, stop=True, tile_position=tp)
                    V = vpool.tile([128, GRP * JO], F32, tag="V")
                    nc.vector.tensor_mul(V[:],
                                         W32_sb[:, grp * GRP * JO:(grp + 1) * GRP * JO],
                                         T_ps[:])
                    V8o = vpool.tile([128, GRP * J], F32, tag="V8o")
                    nc.vector.reduce_sum(
                        out=V8o[:].rearrange("p (c j) -> p c j", c=GRP),
                        in_=V[:].rearrange("p (c j o) -> p c j o", c=GRP, j=J),
                        axis=AX.X)
                    A_ps = ps_a.tile([16, GRP * J], F32, tag="A_ps")
                    nc.tensor.matmul(A_ps[:], selR_sb[:], V8o[:], start=True,
                                     stop=True)
                    nc.vector.tensor_copy(
                        A_sb[:, grp * GRP * J:(grp + 1) * GRP * J], A_ps[:])

                # A_sb[il, (grp,k,j)] -> A_dram[i,j], i = 16*(3*grp+k) + il
                nc.sync.dma_start(
                    A_dram[:].rearrange("(g k l) j -> l g k j", g=NGRP, k=GRP),
                    A_sb[:].rearrange("l (g k j) -> l g k j", g=NGRP, k=GRP))
                nc.gpsimd.collective_compute(
                    "AllReduce", mybir.AluOpType.add,
                    replica_groups=[list(range(NCORES))],
                    ins=[A_dram.opt()], outs=[A_red.opt()])
                nc.sync.dma_start(
                    A_back[:].rearrange("p (c j) -> p c j", c=NCB),
                    A_red[:].rearrange("(c p) j -> p c j", p=128))
                nc.vector.tensor_add(b_sb[:], b_sb[:], A_back[:])

    nc.compile()
    return nc


def _preprocess(x, W):
    """Host-side layout + hi/lo split. Returns per-core in_maps."""
    x = np.ascontiguousarray(x, dtype=np.float32)
    W = np.ascontiguousarray(W, dtype=np.float32)
    Wp = np.ascontiguousarray(W.transpose(0, 3, 1, 2)).reshape(ID, JO)
    Wh = _bf16(Wp)
    Wl = _bf16(Wp - Wh.astype(np.float32))

    sel = np.zeros((8, 128, 128), np.float32)
    for g in range(8):
        for m in range(128):
            sel[g, 16 * g + m // 8, m] = 1.0
    selR = np.zeros((128, 16), np.float32)
    for p in range(128):
        selR[p, p // 8] = 1.0
    rep4 = np.zeros((BL, 128), np.float32)
    for m in range(128):
        rep4[m % BL, m] = 1.0

    shared = {
        "Wp32": Wp,
        "Wp_h": Wh,
        "Wp_l": Wl,
        "sel": _bf16(sel),
        "selR": selR,
        "rep4": _bf16(rep4),
    }
    in_maps = []
    for c in range(NCORES):
        xc = x[c * BL:(c + 1) * BL].reshape(BL, ID)
        xh = _bf16(xc)
        xl = _bf16(xc - xh.astype(np.float32))
        m = dict(shared)
        m["xT_h"] = np.ascontiguousarray(xh.T)
        m["xT_l"] = np.ascontiguousarray(xl.T)
        m["xF_h"] = np.ascontiguousarray(np.tile(xh, (4, 1)))
        m["xF_l"] = np.ascontiguousarray(np.tile(xl, (4, 1)))
        in_maps.append(m)
    return in_maps


def kernel(x, W):
    global LAST_EXEC_NS
    import time
    from concourse.bass_utils import run_bass_kernel_spmd

    if "nc" not in _CACHE:
        _CACHE["nc"] = build_program()
    nc = _CACHE["nc"]

    in_maps = _preprocess(np.asarray(x), np.asarray(W))
    t0 = time.perf_counter()
    res = run_bass_kernel_spmd(nc, in_maps, core_ids=list(range(NCORES)))
    t1 = time.perf_counter()
    LAST_EXEC_NS = res.exec_time_ns
    if LAST_EXEC_NS is None:
        LAST_EXEC_NS = int(1e9 * (t1 - t0))
    _CACHE["last_results"] = res

    out = np.empty((B, J, DOUT), np.float32)
    for c in range(NCORES):
        out[c * BL:(c + 1) * BL] = np.asarray(
            res.results[c]["out_s"], dtype=np.float32).reshape(BL, J, DOUT)
    return out
